# revision 1
# baseline (speedup 1.0000x reference)
"""Trainium2 Bass kernel for 5-sweep Jacobi iteration (4th-order 2D Poisson).

Problem: B=16 samples of [1024,1024] f32; per-sample cross stencil from dx;
5 Jacobi sweeps; 2-wide boundary frame kept fixed at the initial guess.

Sharding: data-parallel over batch, 2 samples per core, 8 cores. Whole
working set (2 x (guess + scaled rhs) = 16 MB) stays resident in SBUF.

Per sweep the stencil is evaluated on the TensorEngine as PSUM-accumulated
matmuls per [128, 512] output unit:
  - banded lhsT   (H-direction taps within the 128-row block)
  - K=2 halo mms  (H taps crossing the block boundary)
  - scaled-identity lhsT with col-shifted rhs APs (W-direction taps)
The VectorEngine then evacuates: new = R + psum (R = dinv * rhs, prescaled
once). Boundary rows/cols are never written, so they keep the initial-guess
values both ping-pong buffers are loaded with.
"""

import sys

sys.path.insert(0, "/opt/trn_rl_repo")

import numpy as np

N_CORES = 8
B, H, W = 16, 1024, 1024
SPC = B // N_CORES  # samples per core
P = 128
NBLK = H // P  # 8 row-blocks
FREE = NBLK * W  # 8192
GRD = 2  # guard cols each side of the g buffers
N_ITER = 5
NHALF = 2  # W halves of 512 (PSUM bank limit for fp32)
MM_DT = "float32r"  # matmul input dtype: float32r | float32 | bfloat16

_CACHE = {}


def _host_coeffs(dx):
    """Per-sample stencil scalars in float64. dx: [B, 2]."""
    a = (1.0 / dx.astype(np.float64)) ** 2
    a0, a1 = a[:, 0], a[:, 1]
    dinv = 1.0 / (-2.5 * (a0 + a1))
    e1 = dinv * a0 * (4.0 / 3.0)
    e2 = dinv * a0 * (-1.0 / 12.0)
    f1 = dinv * a1 * (4.0 / 3.0)
    f2 = dinv * a1 * (-1.0 / 12.0)
    return dinv, e1, e2, f1, f2


def _host_mats(dx):
    """Build [B, 128, 640] lhsT matrices: [Bc | Htop | Hbot | FI1 | FI2].

    All entries are the NEGATED dinv-scaled tap coefficients so that
    psum = -dinv*cr and new = dinv*rhs + psum.
    """
    dinv, e1, e2, f1, f2 = _host_coeffs(dx)
    nb = dx.shape[0]
    mats = np.zeros((nb, P, 5 * P), np.float64)
    idx = np.arange(P)
    for b in range(nb):
        bc = mats[b, :, 0:128]
        for off, v in ((1, -e1[b]), (-1, -e1[b]), (2, -e2[b]), (-2, -e2[b])):
            kk = idx[(idx + off >= 0) & (idx + off < P)]
            bc[kk, kk + off] = v
        ht = mats[b, :, 128:256]
        ht[126, 0] = -e2[b]
        ht[127, 0] = -e1[b]
        ht[127, 1] = -e2[b]
        hb = mats[b, :, 256:384]
        hb[0, 126] = -e2[b]
        hb[0, 127] = -e1[b]
        hb[1, 127] = -e2[b]
        mats[b, :, 384:512][idx, idx] = -f1[b]
        mats[b, :, 512:640][idx, idx] = -f2[b]
    return mats.astype(np.float32), dinv.astype(np.float32)


def _build_nc():
    import concourse.bacc as bacc
    import concourse.tile as tile
    from concourse import mybir

    f32 = mybir.dt.float32
    mm_dt = getattr(mybir.dt, MM_DT)
    nc = bacc.Bacc(
        "TRN2",
        target_bir_lowering=False,
        debug=False,
        enable_asserts=False,
        num_devices=N_CORES,
    )
    g_d = nc.dram_tensor("g", [SPC, P, FREE + 2 * GRD], mm_dt, kind="ExternalInput").ap()
    r_d = nc.dram_tensor("r", [SPC, P, FREE], f32, kind="ExternalInput").ap()
    m_d = nc.dram_tensor("m", [SPC, P, 5 * P], mm_dt, kind="ExternalInput").ap()
    c_d = nc.dram_tensor("c", [SPC, P, 1], f32, kind="ExternalInput").ap()
    o_d = nc.dram_tensor("o", [SPC, P, FREE], f32, kind="ExternalOutput").ap()

    with tile.TileContext(nc) as tc:
        with (
            tc.tile_pool(name="state", bufs=1) as state,
            tc.tile_pool(name="psum", bufs=8, space="PSUM") as pp,
        ):
            gb = [
                [state.tile([P, FREE + 2 * GRD], mm_dt, name=f"g{s}_{i}", tag=f"g{s}_{i}") for i in range(2)]
                for s in range(SPC)
            ]
            rb = [state.tile([P, FREE], f32, name=f"r{s}", tag=f"r{s}") for s in range(SPC)]
            mt = [state.tile([P, 5 * P], mm_dt, name=f"m{s}", tag=f"m{s}") for s in range(SPC)]
            cf = [state.tile([P, 1], f32, name=f"c{s}", tag=f"c{s}") for s in range(SPC)]

            for s in range(SPC):
                for i in range(2):
                    nc.sync.dma_start(gb[s][i][:], g_d[s])
                nc.sync.dma_start(rb[s][:], r_d[s])
                nc.sync.dma_start(mt[s][:], m_d[s])
                nc.sync.dma_start(cf[s][:], c_d[s])
                # R = dinv * rhs, in place
                nc.vector.tensor_scalar_mul(rb[s][:], rb[s][:], cf[s][:, 0:1])

            for it in range(N_ITER):
                for s in range(SPC):
                    cur = gb[s][it % 2]
                    nxt = gb[s][(it + 1) % 2]
                    for k in range(NBLK):
                        for xh in range(NHALF):
                            ps = pp.tile([P, 512], f32, name="ps", tag="ps")
                            cb = GRD + W * k + 512 * xh
                            mms = []
                            # H main: banded Bc
                            mms.append((mt[s][:, 0:128], cur[:, cb : cb + 512], None))
                            # W shifts: FI1 (+-1), FI2 (+-2)
                            for mcol, d in ((384, -1), (384, 1), (512, -2), (512, 2)):
                                mms.append(
                                    (
                                        mt[s][:, mcol : mcol + 128],
                                        cur[:, cb + d : cb + d + 512],
                                        None,
                                    )
                                )
                            # halo mms: full K=128 with zero-padded lhsT rows
                            # (K-subtiling via tile_position crashes the device)
                            if k > 0:  # prev block rows 126,127 -> out rows 0,1
                                mms.append(
                                    (mt[s][:, 128:256], cur[:, cb - W : cb - W + 512], None)
                                )
                            if k < NBLK - 1:  # next block rows 0,1 -> out 126,127
                                mms.append(
                                    (mt[s][:, 256:384], cur[:, cb + W : cb + W + 512], None)
                                )
                            for j, (lhsT, rhs, tpos) in enumerate(mms):
                                nc.tensor.matmul(
                                    ps[:, :],
                                    lhsT,
                                    rhs,
                                    start=(j == 0),
                                    stop=(j == len(mms) - 1),
                                    skip_group_check=True,
                                    tile_position=tpos,
                                )
                            # evacuate: new = R + psum. Partition bases must be
                            # quadrant-aligned, so block 7 stops at row 126 and
                            # block 0 evacuates rows 0,1 too (restored below).
                            p1 = 126 if k == NBLK - 1 else 128
                            n0 = 2 if xh == 0 else 0
                            n1 = 510 if xh == NHALF - 1 else 512
                            rbase = W * k + 512 * xh
                            nc.vector.tensor_add(
                                nxt[0:p1, cb + n0 : cb + n1],
                                ps[0:p1, n0:n1],
                                rb[s][0:p1, rbase + n0 : rbase + n1],
                            )
                        # restore boundary rows 0,1 clobbered by the full evac
                        if k == 0:
                            nc.scalar.copy(
                                nxt[0:2, GRD : GRD + W], cur[0:2, GRD : GRD + W]
                            )

            for s in range(SPC):
                final = gb[s][N_ITER % 2]
                nc.sync.dma_start(o_d[s], final[:, GRD : GRD + FREE].bitcast(f32))

    nc.compile()
    return nc


def _get_nc():
    if "nc" not in _CACHE:
        _CACHE["nc"] = _build_nc()
    return _CACHE["nc"]


def _round_f32r(x):
    """Round fp32 to float32r precision (11 explicit mantissa bits, RNE)."""
    if MM_DT != "float32r":
        return x
    b = np.ascontiguousarray(x, dtype=np.float32).view(np.uint32)
    drop = 12
    lsb = (b >> drop) & np.uint32(1)
    b = (b + np.uint32((1 << (drop - 1)) - 1) + lsb) & np.uint32(~((1 << drop) - 1) & 0xFFFFFFFF)
    return b.view(np.float32)


def _to_block(x):
    """[B, H, W] -> [B, P, FREE]: out[b, p, k*W + x] = in[b, 128k+p, x]."""
    nb = x.shape[0]
    return np.ascontiguousarray(
        x.reshape(nb, NBLK, P, W).transpose(0, 2, 1, 3).reshape(nb, P, FREE)
    )


def _from_block(x):
    nb = x.shape[0]
    return np.ascontiguousarray(
        x.reshape(nb, P, NBLK, W).transpose(0, 2, 1, 3).reshape(nb, H, W)
    )


def kernel(current_guess, rhses, dx):
    from concourse.bass_utils import run_bass_kernel_spmd

    g = _round_f32r(_to_block(np.ascontiguousarray(current_guess[:, 0], dtype=np.float32)))
    gpad = np.zeros((B, P, FREE + 2 * GRD), np.float32)
    gpad[:, :, GRD : GRD + FREE] = g
    g = gpad
    r = _to_block(np.ascontiguousarray(rhses[:, 0], dtype=np.float32))
    mats, dinv = _host_mats(dx)
    mats = _round_f32r(mats)
    coef = np.ascontiguousarray(
        np.broadcast_to(dinv[:, None, None], (B, P, 1)), dtype=np.float32
    )

    nc = _get_nc()
    in_maps = []
    for c in range(N_CORES):
        sl = slice(c * SPC, (c + 1) * SPC)
        in_maps.append(
            {
                "g": np.ascontiguousarray(g[sl]),
                "r": np.ascontiguousarray(r[sl]),
                "m": np.ascontiguousarray(mats[sl]),
                "c": np.ascontiguousarray(coef[sl]),
            }
        )
    res = run_bass_kernel_spmd(nc, in_maps, core_ids=list(range(N_CORES)))
    _CACHE["last_results"] = res
    ob = np.concatenate([res.results[c]["o"] for c in range(N_CORES)], axis=0)
    return _from_block(ob)[:, None].astype(np.float32)



# revision 2
# speedup vs baseline: 1.4372x; 1.4372x over previous
"""Trainium2 Bass kernel for 5-sweep Jacobi iteration (4th-order 2D Poisson).

Problem: B=16 samples of [1024,1024] f32; per-sample cross stencil from dx;
5 Jacobi sweeps; 2-wide boundary frame kept fixed at the initial guess.

Sharding: data-parallel over batch, 2 samples per core, 8 cores.

Layout: bf16 state, 9 row-blocks of 128 rows overlapping by 4 rows
(block b holds rows 124b..124b+128; block 8 holds rows 896..1024). Each
block computes out rows [2,126) locally (block 8: [98,126)) so the
H-direction taps never cross a block boundary -> no halo matmuls. The
4-row overlaps are kept coherent with small SBUF->SBUF DMAs per sweep.

Per [128,512] output unit and sweep:
  PE   : psum = Bc@x (H taps) + (-f1)(x<<1 + x>>1) + I@R     (4 matmuls)
  DVE/GPS: A2 = x<<2 + x>>2                                  (tensor_add)
  DVE  : nxt = (A2 * -f2) + psum                             (fused STT)
Boundary cols are never written (col-trimmed evac); boundary rows are
restored by DMA; host splices the exact fp32 boundary frame at the end.
"""

import sys

sys.path.insert(0, "/opt/trn_rl_repo")

import numpy as np
import ml_dtypes

BF = ml_dtypes.bfloat16

N_CORES = 8
B, H, W = 16, 1024, 1024
SPC = B // N_CORES  # samples per core
P = 128
OPB = 124            # out rows per block
NBLK = 9             # row blocks (8 full-stride + 1 tail)
BW = W + 4           # block width incl 2 guard cols each side
FREE = NBLK * BW     # 9252
RFREE = NBLK * W     # 9216
N_ITER = 5
GPS_FRAC_NUM, GPS_FRAC_DEN = 2, 5   # ~40% of A2 adds on gpsimd

_CACHE = {}


def _row_start(b):
    return 124 * b if b < NBLK - 1 else H - P  # block 8: rows 896..1024


def _host_coeffs(dx):
    """Per-sample stencil scalars in float64. dx: [B, 2]."""
    a = (1.0 / dx.astype(np.float64)) ** 2
    a0, a1 = a[:, 0], a[:, 1]
    dinv = 1.0 / (-2.5 * (a0 + a1))
    e1 = dinv * a0 * (4.0 / 3.0)
    e2 = dinv * a0 * (-1.0 / 12.0)
    f1 = dinv * a1 * (4.0 / 3.0)
    f2 = dinv * a1 * (-1.0 / 12.0)
    return dinv, e1, e2, f1, f2


def _host_mats(dx):
    """[B, 128, 384] lhsT mats: [Bc(-e taps) | -f1*I | I], plus -f2 scalars."""
    dinv, e1, e2, f1, f2 = _host_coeffs(dx)
    nb = dx.shape[0]
    mats = np.zeros((nb, P, 3 * P), np.float64)
    idx = np.arange(P)
    for s in range(nb):
        bc = mats[s, :, 0:P]
        for off, v in ((1, -e1[s]), (-1, -e1[s]), (2, -e2[s]), (-2, -e2[s])):
            kk = idx[(idx + off >= 0) & (idx + off < P)]
            bc[kk, kk + off] = v
        mats[s, :, P:2 * P][idx, idx] = -f1[s]
        mats[s, :, 2 * P:3 * P][idx, idx] = 1.0
    scal = np.broadcast_to((-f2)[:, None, None], (nb, P, 1))
    return mats.astype(BF), np.ascontiguousarray(scal, dtype=np.float32), dinv


def _build_nc():
    import concourse.bacc as bacc
    import concourse.tile as tile
    from concourse import mybir

    f32 = mybir.dt.float32
    bf16 = mybir.dt.bfloat16
    nc = bacc.Bacc(
        "TRN2",
        target_bir_lowering=False,
        debug=False,
        enable_asserts=False,
        num_devices=N_CORES,
    )
    g_d = nc.dram_tensor("g", [SPC, P, FREE], bf16, kind="ExternalInput").ap()
    r_d = nc.dram_tensor("r", [SPC, P, RFREE], bf16, kind="ExternalInput").ap()
    m_d = nc.dram_tensor("m", [SPC, P, 3 * P], bf16, kind="ExternalInput").ap()
    c_d = nc.dram_tensor("c", [SPC, P, 1], f32, kind="ExternalInput").ap()
    o_d = nc.dram_tensor("o", [SPC, P, FREE], bf16, kind="ExternalOutput").ap()

    with tile.TileContext(nc) as tc:
        with (
            tc.tile_pool(name="state", bufs=1) as state,
            tc.tile_pool(name="tmp", bufs=6) as tmp,
            tc.tile_pool(name="psum", bufs=8, space="PSUM") as pp,
        ):
            gb = [
                [state.tile([P, FREE], bf16, name=f"g{s}_{i}", tag=f"g{s}_{i}")
                 for i in range(2)]
                for s in range(SPC)
            ]
            rb = [state.tile([P, RFREE], bf16, name=f"r{s}", tag=f"r{s}")
                  for s in range(SPC)]
            mt = [state.tile([P, 3 * P], bf16, name=f"m{s}", tag=f"m{s}")
                  for s in range(SPC)]
            cf = [state.tile([P, 1], f32, name=f"c{s}", tag=f"c{s}")
                  for s in range(SPC)]

            for s in range(SPC):
                for b in range(NBLK):
                    for i in range(2):
                        nc.sync.dma_start(
                            gb[s][i][:, BW * b: BW * (b + 1)],
                            g_d[s][:, BW * b: BW * (b + 1)])
                    nc.sync.dma_start(
                        rb[s][:, W * b: W * (b + 1)],
                        r_d[s][:, W * b: W * (b + 1)])
                nc.sync.dma_start(mt[s][:], m_d[s])
                nc.sync.dma_start(cf[s][:], c_d[s])

            uidx = 0
            for it in range(N_ITER):
                for s in range(SPC):
                    cur = gb[s][it % 2]
                    nxt = gb[s][(it + 1) % 2]
                    for b in range(NBLK):
                        for h2 in range(2):
                            ps = pp.tile([P, 512], f32, name="ps", tag="ps")
                            a2 = tmp.tile([P, 512], bf16, name="a2", tag="a2")
                            base = BW * b + 2 + 512 * h2
                            # A2 = x<<2 + x>>2 on DVE or GPSIMD
                            eng = (nc.gpsimd
                                   if uidx % GPS_FRAC_DEN < GPS_FRAC_NUM
                                   else nc.vector)
                            eng.tensor_add(a2[:], cur[:, base - 2: base + 510],
                                           cur[:, base + 2: base + 514])
                            uidx += 1
                            # PE: H banded + f1 shifts + R inject
                            nc.tensor.matmul(ps[:], mt[s][:, 0:P],
                                             cur[:, base: base + 512],
                                             start=True, stop=False,
                                             skip_group_check=True)
                            for d in (-1, 1):
                                nc.tensor.matmul(ps[:], mt[s][:, P:2 * P],
                                                 cur[:, base + d: base + d + 512],
                                                 start=False, stop=False,
                                                 skip_group_check=True)
                            nc.tensor.matmul(ps[:], mt[s][:, 2 * P:3 * P],
                                             rb[s][:, W * b + 512 * h2:
                                                   W * b + 512 * h2 + 512],
                                             start=False, stop=True,
                                             skip_group_check=True)
                            # evac: nxt = (A2 * -f2) + psum
                            n0 = 2 if h2 == 0 else 0
                            n1 = 510 if h2 == 1 else 512
                            p0, psz = (96, 30) if b == NBLK - 1 else (0, 126)
                            nc.vector.scalar_tensor_tensor(
                                nxt[p0:p0 + psz, base + n0: base + n1],
                                a2[p0:p0 + psz, n0:n1],
                                cf[s][p0:p0 + psz, 0:1],
                                ps[p0:p0 + psz, n0:n1],
                                op0=mybir.AluOpType.mult,
                                op1=mybir.AluOpType.add,
                            )
                    # overlap-row maintenance for next sweep
                    for bd in range(NBLK - 1):
                        b1 = bd + 1
                        u0 = 0 if bd < NBLK - 2 else 96
                        d0 = 2 if bd < NBLK - 2 else 98
                        nc.sync.dma_start(
                            nxt[u0:u0 + 2, BW * b1: BW * (b1 + 1)],
                            nxt[124:126, BW * bd: BW * (bd + 1)])
                        nc.sync.dma_start(
                            nxt[126:128, BW * bd: BW * (bd + 1)],
                            nxt[d0:d0 + 2, BW * b1: BW * (b1 + 1)])
                    # restore fixed global rows 0,1 (block 0)
                    nc.sync.dma_start(nxt[0:2, 0:BW], cur[0:2, 0:BW])

            for s in range(SPC):
                final = gb[s][N_ITER % 2]
                for b in range(NBLK):
                    nc.sync.dma_start(o_d[s][:, BW * b: BW * (b + 1)],
                                      final[:, BW * b: BW * (b + 1)])

    nc.compile()
    return nc


def _get_nc():
    if "nc" not in _CACHE:
        _CACHE["nc"] = _build_nc()
    return _CACHE["nc"]


def _to_blocks(x, width, guard):
    """[B, H, W(+0)] f32 -> [B, P, NBLK*(W+2*guard)] bf16 with row overlap."""
    nb = x.shape[0]
    out = np.zeros((nb, P, NBLK * (width + 2 * guard)), BF)
    for b in range(NBLK):
        rs = _row_start(b)
        sl = out[:, :, b * (width + 2 * guard) + guard:
                 (b + 1) * (width + 2 * guard) - guard]
        sl[:] = x[:, rs:rs + P, :].astype(BF)
    return out


def kernel(current_guess, rhses, dx):
    from concourse.bass_utils import run_bass_kernel_spmd

    g32 = np.ascontiguousarray(current_guess[:, 0], dtype=np.float32)
    r32 = np.ascontiguousarray(rhses[:, 0], dtype=np.float32)
    mats, scal, dinv = _host_mats(dx)
    g = _to_blocks(g32, W, 2)
    r = _to_blocks(r32 * dinv[:, None, None].astype(np.float32), W, 0)

    nc = _get_nc()
    in_maps = []
    for c in range(N_CORES):
        sl = slice(c * SPC, (c + 1) * SPC)
        in_maps.append({
            "g": np.ascontiguousarray(g[sl]).view(np.uint16),
            "r": np.ascontiguousarray(r[sl]).view(np.uint16),
            "m": np.ascontiguousarray(mats[sl]).view(np.uint16),
            "c": np.ascontiguousarray(scal[sl]),
        })
    res = run_bass_kernel_spmd(nc, in_maps, core_ids=list(range(N_CORES)))
    _CACHE["last_results"] = res
    ob = np.concatenate([res.results[c]["o"] for c in range(N_CORES)], axis=0)
    blk = ob.view(BF).astype(np.float32).reshape(B, P, NBLK, BW).transpose(0, 2, 1, 3)

    out = np.empty((B, H, W), np.float32)
    for b in range(NBLK - 1):
        out[:, 124 * b + 2: 124 * b + 126, :] = blk[:, b, 2:126, 2:2 + W]
    out[:, 994:1022, :] = blk[:, NBLK - 1, 98:126, 2:2 + W]
    # exact fp32 boundary frame from the input
    out[:, 0:2, :] = g32[:, 0:2, :]
    out[:, 1022:1024, :] = g32[:, 1022:1024, :]
    out[:, :, 0:2] = g32[:, :, 0:2]
    out[:, :, 1022:1024] = g32[:, :, 1022:1024]
    return out[:, None].astype(np.float32)


# revision 7
# speedup vs baseline: 1.4488x; 1.0081x over previous
"""Trainium2 Bass kernel for 5-sweep Jacobi iteration (4th-order 2D Poisson).

Problem: B=16 samples of [1024,1024] f32; per-sample cross stencil from dx;
5 Jacobi sweeps; 2-wide boundary frame kept fixed at the initial guess.

Sharding: data-parallel over batch, 2 samples per core, 8 cores.

Layout: bf16 state, 9 row-blocks of 128 rows overlapping by 4 rows
(block b holds rows 124b..124b+128; block 8 holds rows 896..1024). Each
block computes out rows [2,126) locally (block 8: [98,126)) so the
H-direction taps never cross a block boundary -> no halo matmuls. The
4-row overlaps are kept coherent with small SBUF->SBUF DMAs per sweep.

Per [128,512] output unit and sweep:
  PE   : psum = Bc@x (H taps) + (-f1)(x<<1 + x>>1) + I@R     (4 matmuls)
  DVE/GPS: A2 = x<<2 + x>>2                                  (tensor_add)
  DVE  : nxt = (A2 * -f2) + psum                             (fused STT)
Boundary cols are never written (col-trimmed evac); boundary rows are
restored by DMA; host splices the exact fp32 boundary frame at the end.
"""

import sys

sys.path.insert(0, "/opt/trn_rl_repo")

import numpy as np
import ml_dtypes

BF = ml_dtypes.bfloat16

N_CORES = 8
B, H, W = 16, 1024, 1024
SPC = B // N_CORES  # samples per core
P = 128
OPB = 124            # out rows per block
NBLK = 9             # row blocks (8 full-stride + 1 tail)
BW = W + 4           # block width incl 2 guard cols each side
FREE = NBLK * BW     # 9252
RFREE = NBLK * W     # 9216
N_ITER = 5
GPS_FRAC_NUM, GPS_FRAC_DEN = 2, 5   # ~40% of A2 adds on gpsimd

_CACHE = {}


def _row_start(b):
    return 124 * b if b < NBLK - 1 else H - P  # block 8: rows 896..1024


def _host_coeffs(dx):
    """Per-sample stencil scalars in float64. dx: [B, 2]."""
    a = (1.0 / dx.astype(np.float64)) ** 2
    a0, a1 = a[:, 0], a[:, 1]
    dinv = 1.0 / (-2.5 * (a0 + a1))
    e1 = dinv * a0 * (4.0 / 3.0)
    e2 = dinv * a0 * (-1.0 / 12.0)
    f1 = dinv * a1 * (4.0 / 3.0)
    f2 = dinv * a1 * (-1.0 / 12.0)
    return dinv, e1, e2, f1, f2


def _host_mats(dx):
    """[B, 128, 384] lhsT mats: [Bc(-e taps) | -f1*I | I], plus -f2 scalars."""
    dinv, e1, e2, f1, f2 = _host_coeffs(dx)
    nb = dx.shape[0]
    mats = np.zeros((nb, P, 3 * P), np.float64)
    idx = np.arange(P)
    for s in range(nb):
        bc = mats[s, :, 0:P]
        for off, v in ((1, -e1[s]), (-1, -e1[s]), (2, -e2[s]), (-2, -e2[s])):
            kk = idx[(idx + off >= 0) & (idx + off < P)]
            bc[kk, kk + off] = v
        mats[s, :, P:2 * P][idx, idx] = -f1[s]
        mats[s, :, 2 * P:3 * P][idx, idx] = 1.0
    scal = np.broadcast_to((-f2)[:, None, None], (nb, P, 1))
    return mats.astype(BF), np.ascontiguousarray(scal, dtype=np.float32), dinv


def _build_nc():
    import concourse.bacc as bacc
    import concourse.tile as tile
    from concourse import mybir

    f32 = mybir.dt.float32
    bf16 = mybir.dt.bfloat16
    nc = bacc.Bacc(
        "TRN2",
        target_bir_lowering=False,
        debug=False,
        enable_asserts=False,
        num_devices=N_CORES,
    )
    g_d = nc.dram_tensor("g", [SPC, P, FREE], bf16, kind="ExternalInput").ap()
    r_d = nc.dram_tensor("r", [SPC, P, RFREE], bf16, kind="ExternalInput").ap()
    m_d = nc.dram_tensor("m", [SPC, P, 3 * P], bf16, kind="ExternalInput").ap()
    c_d = nc.dram_tensor("c", [SPC, P, 1], f32, kind="ExternalInput").ap()
    o_d = nc.dram_tensor("o", [SPC, P, FREE], bf16, kind="ExternalOutput").ap()

    with tile.TileContext(nc) as tc:
        with (
            tc.tile_pool(name="state", bufs=1) as state,
            tc.tile_pool(name="tmp", bufs=6) as tmp,
            tc.tile_pool(name="psum", bufs=4, space="PSUM") as pp,
        ):
            gb = [
                [state.tile([P, FREE], bf16, name=f"g{s}_{i}", tag=f"g{s}_{i}")
                 for i in range(2)]
                for s in range(SPC)
            ]
            rb = [state.tile([P, RFREE], bf16, name=f"r{s}", tag=f"r{s}")
                  for s in range(SPC)]
            mt = [state.tile([P, 3 * P], bf16, name=f"m{s}", tag=f"m{s}")
                  for s in range(SPC)]
            cf = [state.tile([P, 1], f32, name=f"c{s}", tag=f"c{s}")
                  for s in range(SPC)]

            for s in range(SPC):
                for b in range(NBLK):
                    for i in range(2):
                        nc.sync.dma_start(
                            gb[s][i][:, BW * b: BW * (b + 1)],
                            g_d[s][:, BW * b: BW * (b + 1)])
                    nc.sync.dma_start(
                        rb[s][:, W * b: W * (b + 1)],
                        r_d[s][:, W * b: W * (b + 1)])
                nc.sync.dma_start(mt[s][:], m_d[s])
                nc.sync.dma_start(cf[s][:], c_d[s])

            bidx = 0
            for it in range(N_ITER):
                for s in range(SPC):
                    cur = gb[s][it % 2]
                    nxt = gb[s][(it + 1) % 2]
                    for b in range(NBLK):
                        ps = pp.tile([P, 1024], f32, name="ps", tag="ps")
                        a2 = tmp.tile([P, 1020], bf16, name="a2", tag="a2")
                        bof = BW * b
                        # A2[n] = x[c-2]+x[c+2], c=n+2, on DVE or GPSIMD
                        eng = (nc.gpsimd if bidx % 9 in (1, 3, 5, 7)
                               else nc.vector)
                        eng.tensor_add(a2[:], cur[:, bof + 2: bof + 1022],
                                       cur[:, bof + 6: bof + 1026])
                        bidx += 1
                        # PE: H banded + f1 shifts + R inject, per 512-half
                        for h2 in range(2):
                            base = bof + 2 + 512 * h2
                            po = 512 * h2
                            nc.tensor.matmul(ps[:, po:po + 512], mt[s][:, 0:P],
                                             cur[:, base: base + 512],
                                             start=True, stop=False,
                                             skip_group_check=True)
                            for d in (-1, 1):
                                nc.tensor.matmul(ps[:, po:po + 512],
                                                 mt[s][:, P:2 * P],
                                                 cur[:, base + d: base + d + 512],
                                                 start=False, stop=False,
                                                 skip_group_check=True)
                            nc.tensor.matmul(ps[:, po:po + 512],
                                             mt[s][:, 2 * P:3 * P],
                                             rb[s][:, W * b + po:
                                                   W * b + po + 512],
                                             start=False, stop=True,
                                             skip_group_check=True)
                        # evac whole block: nxt = (A2 * -f2) + psum
                        p0, psz = (96, 30) if b == NBLK - 1 else (0, 126)
                        nc.vector.scalar_tensor_tensor(
                            nxt[p0:p0 + psz, bof + 4: bof + 1024],
                            a2[p0:p0 + psz, 0:1020],
                            cf[s][p0:p0 + psz, 0:1],
                            ps[p0:p0 + psz, 2:1022],
                            op0=mybir.AluOpType.mult,
                            op1=mybir.AluOpType.add,
                        )
                    # overlap-row maintenance for next sweep
                    for bd in range(NBLK - 1):
                        b1 = bd + 1
                        u0 = 0 if bd < NBLK - 2 else 96
                        d0 = 2 if bd < NBLK - 2 else 98
                        nc.sync.dma_start(
                            nxt[u0:u0 + 2, BW * b1: BW * (b1 + 1)],
                            nxt[124:126, BW * bd: BW * (bd + 1)])
                        nc.scalar.dma_start(
                            nxt[126:128, BW * bd: BW * (bd + 1)],
                            nxt[d0:d0 + 2, BW * b1: BW * (b1 + 1)])
                    # restore fixed global rows 0,1 (block 0)
                    nc.scalar.dma_start(nxt[0:2, 0:BW], cur[0:2, 0:BW])

            for s in range(SPC):
                final = gb[s][N_ITER % 2]
                for b in range(NBLK):
                    nc.sync.dma_start(o_d[s][:, BW * b: BW * (b + 1)],
                                      final[:, BW * b: BW * (b + 1)])

    nc.compile()
    return nc


def _get_nc():
    if "nc" not in _CACHE:
        _CACHE["nc"] = _build_nc()
    return _CACHE["nc"]


def _to_blocks(x, width, guard):
    """[B, H, W(+0)] f32 -> [B, P, NBLK*(W+2*guard)] bf16 with row overlap."""
    nb = x.shape[0]
    out = np.zeros((nb, P, NBLK * (width + 2 * guard)), BF)
    for b in range(NBLK):
        rs = _row_start(b)
        sl = out[:, :, b * (width + 2 * guard) + guard:
                 (b + 1) * (width + 2 * guard) - guard]
        sl[:] = x[:, rs:rs + P, :].astype(BF)
    return out


def kernel(current_guess, rhses, dx):
    from concourse.bass_utils import run_bass_kernel_spmd

    g32 = np.ascontiguousarray(current_guess[:, 0], dtype=np.float32)
    r32 = np.ascontiguousarray(rhses[:, 0], dtype=np.float32)
    mats, scal, dinv = _host_mats(dx)
    g = _to_blocks(g32, W, 2)
    r = _to_blocks(r32 * dinv[:, None, None].astype(np.float32), W, 0)

    nc = _get_nc()
    in_maps = []
    for c in range(N_CORES):
        sl = slice(c * SPC, (c + 1) * SPC)
        in_maps.append({
            "g": np.ascontiguousarray(g[sl]).view(np.uint16),
            "r": np.ascontiguousarray(r[sl]).view(np.uint16),
            "m": np.ascontiguousarray(mats[sl]).view(np.uint16),
            "c": np.ascontiguousarray(scal[sl]),
        })
    res = run_bass_kernel_spmd(nc, in_maps, core_ids=list(range(N_CORES)))
    _CACHE["last_results"] = res
    ob = np.concatenate([res.results[c]["o"] for c in range(N_CORES)], axis=0)
    blk = ob.view(BF).astype(np.float32).reshape(B, P, NBLK, BW).transpose(0, 2, 1, 3)

    out = np.empty((B, H, W), np.float32)
    for b in range(NBLK - 1):
        out[:, 124 * b + 2: 124 * b + 126, :] = blk[:, b, 2:126, 2:2 + W]
    out[:, 994:1022, :] = blk[:, NBLK - 1, 98:126, 2:2 + W]
    # exact fp32 boundary frame from the input
    out[:, 0:2, :] = g32[:, 0:2, :]
    out[:, 1022:1024, :] = g32[:, 1022:1024, :]
    out[:, :, 0:2] = g32[:, :, 0:2]
    out[:, :, 1022:1024] = g32[:, :, 1022:1024]
    return out[:, None].astype(np.float32)


# revision 12
# speedup vs baseline: 1.5079x; 1.0408x over previous
"""Trainium2 Bass kernel for 5-sweep Jacobi iteration (4th-order 2D Poisson).

Problem: B=16 samples of [1024,1024] f32; per-sample cross stencil from dx;
5 Jacobi sweeps; 2-wide boundary frame kept fixed at the initial guess.

Sharding: data-parallel over batch, 2 samples per core, 8 cores.

Layout: bf16 state, 9 row-blocks of 128 rows overlapping by 4 rows
(block b holds rows 124b..124b+128; block 8 holds rows 896..1024). Each
block computes out rows [2,126) locally (block 8: [98,126)) so the
H-direction taps never cross a block boundary -> no halo matmuls. The
4-row overlaps are kept coherent with small SBUF->SBUF DMAs per sweep.

Per [128,512] output unit and sweep:
  PE   : psum = Bc@x (H taps) + (-f1)(x<<1 + x>>1) + I@R     (4 matmuls)
  DVE/GPS: A2 = x<<2 + x>>2                                  (tensor_add)
  DVE  : nxt = (A2 * -f2) + psum                             (fused STT)
Boundary cols are never written (col-trimmed evac); boundary rows are
restored by DMA; host splices the exact fp32 boundary frame at the end.
"""

import sys

sys.path.insert(0, "/opt/trn_rl_repo")

import numpy as np
import ml_dtypes

BF = ml_dtypes.bfloat16

N_CORES = 8
B, H, W = 16, 1024, 1024
SPC = B // N_CORES  # samples per core
P = 128
OPB = 124            # out rows per block
NBLK = 9             # row blocks (8 full-stride + 1 tail)
BW = W + 4           # block width incl 2 guard cols each side
FREE = NBLK * BW     # 9252
RFREE = NBLK * W     # 9216
N_ITER = 5
GPS_FRAC_NUM, GPS_FRAC_DEN = 2, 5   # ~40% of A2 adds on gpsimd

_CACHE = {}


def _row_start(b):
    return 124 * b if b < NBLK - 1 else H - P  # block 8: rows 896..1024


def _host_coeffs(dx):
    """Per-sample stencil scalars in float64. dx: [B, 2]."""
    a = (1.0 / dx.astype(np.float64)) ** 2
    a0, a1 = a[:, 0], a[:, 1]
    dinv = 1.0 / (-2.5 * (a0 + a1))
    e1 = dinv * a0 * (4.0 / 3.0)
    e2 = dinv * a0 * (-1.0 / 12.0)
    f1 = dinv * a1 * (4.0 / 3.0)
    f2 = dinv * a1 * (-1.0 / 12.0)
    return dinv, e1, e2, f1, f2


def _host_mats(dx):
    """[B, 128, 384] lhsT mats: [Bc(-e taps) | -f1*I | I], plus -f2 scalars."""
    dinv, e1, e2, f1, f2 = _host_coeffs(dx)
    nb = dx.shape[0]
    mats = np.zeros((nb, P, 3 * P), np.float64)
    idx = np.arange(P)
    for s in range(nb):
        bc = mats[s, :, 0:P]
        for off, v in ((1, -e1[s]), (-1, -e1[s]), (2, -e2[s]), (-2, -e2[s])):
            kk = idx[(idx + off >= 0) & (idx + off < P)]
            bc[kk, kk + off] = v
        mats[s, :, P:2 * P][idx, idx] = -f1[s]
        mats[s, :, 2 * P:3 * P][idx, idx] = 1.0
    scal = np.broadcast_to((-f2)[:, None, None], (nb, P, 1))
    return mats.astype(BF), np.ascontiguousarray(scal, dtype=np.float32), dinv


def _build_nc():
    import concourse.bacc as bacc
    import concourse.tile as tile
    from concourse import mybir

    f32 = mybir.dt.float32
    bf16 = mybir.dt.bfloat16
    nc = bacc.Bacc(
        "TRN2",
        target_bir_lowering=False,
        debug=False,
        enable_asserts=False,
        num_devices=N_CORES,
    )
    g_d = nc.dram_tensor("g", [SPC, P, FREE], bf16, kind="ExternalInput").ap()
    r_d = nc.dram_tensor("r", [SPC, P, RFREE], bf16, kind="ExternalInput").ap()
    m_d = nc.dram_tensor("m", [SPC, P, 3 * P], bf16, kind="ExternalInput").ap()
    c_d = nc.dram_tensor("c", [SPC, P, 1], f32, kind="ExternalInput").ap()
    o_d = nc.dram_tensor("o", [SPC, P, FREE], bf16, kind="ExternalOutput").ap()

    with tile.TileContext(nc) as tc:
        with (
            tc.tile_pool(name="state", bufs=1) as state,
            tc.tile_pool(name="tmp", bufs=6) as tmp,
            tc.tile_pool(name="psum", bufs=4, space="PSUM") as pp,
        ):
            gb = [
                [state.tile([P, FREE], bf16, name=f"g{s}_{i}", tag=f"g{s}_{i}")
                 for i in range(2)]
                for s in range(SPC)
            ]
            rb = [state.tile([P, RFREE], bf16, name=f"r{s}", tag=f"r{s}")
                  for s in range(SPC)]
            mt = [state.tile([P, 3 * P], bf16, name=f"m{s}", tag=f"m{s}")
                  for s in range(SPC)]
            cf = [state.tile([P, 1], f32, name=f"c{s}", tag=f"c{s}")
                  for s in range(SPC)]

            from concourse.ap import AP

            for s in range(SPC):
                nc.sync.dma_start(mt[s][:], m_d[s])
                nc.sync.dma_start(cf[s][:], c_d[s])
            for s in range(SPC):
                for b3 in range(3):  # batched thirds, sample-major
                    lo, hi = 3 * b3, 3 * (b3 + 1)
                    nc.sync.dma_start(gb[s][0][:, BW * lo: BW * hi],
                                      g_d[s][:, BW * lo: BW * hi])
                    nc.scalar.dma_start(rb[s][:, W * lo: W * hi],
                                        r_d[s][:, W * lo: W * hi])
            for s in range(SPC):
                # buffer 1 never gets a full load: evac rewrites everything
                # except the fixed boundary-col strips (offsets 2,3 / 1024,
                # 1025 per block) and block-8 rows 126,127.
                g0 = gb[s][0][:]
                g1 = gb[s][1][:]
                strips = [[9252, 128], [1028, NBLK], [1, 2]]
                for so in (2, 1024):
                    nc.sync.dma_start(
                        AP(tensor=g1.tensor, offset=g1.offset + so, ap=strips),
                        AP(tensor=g0.tensor, offset=g0.offset + so, ap=strips))
                # block 8 fully: partitions 0..95 are never evac'd and NaN
                # garbage there would poison the contraction
                nc.sync.dma_start(
                    gb[s][1][:, BW * (NBLK - 1): BW * NBLK],
                    gb[s][0][:, BW * (NBLK - 1): BW * NBLK])

            bidx = 0
            for it in range(N_ITER):
                for s in range(SPC):
                    cur = gb[s][it % 2]
                    nxt = gb[s][(it + 1) % 2]
                    for b in range(NBLK):
                        ps = pp.tile([P, 1024], f32, name="ps", tag="ps")
                        a2 = tmp.tile([P, 1020], bf16, name="a2", tag="a2")
                        bof = BW * b
                        # A2[n] = x[c-2]+x[c+2], c=n+2, on DVE or GPSIMD
                        eng = (nc.gpsimd if bidx % 9 in (1, 3, 5, 7)
                               else nc.vector)
                        eng.tensor_add(a2[:], cur[:, bof + 2: bof + 1022],
                                       cur[:, bof + 6: bof + 1026])
                        bidx += 1
                        # PE: H banded + f1 shifts + R inject, per 512-half
                        for h2 in range(2):
                            base = bof + 2 + 512 * h2
                            po = 512 * h2
                            nc.tensor.matmul(ps[:, po:po + 512], mt[s][:, 0:P],
                                             cur[:, base: base + 512],
                                             start=True, stop=False,
                                             skip_group_check=True)
                            for d in (-1, 1):
                                nc.tensor.matmul(ps[:, po:po + 512],
                                                 mt[s][:, P:2 * P],
                                                 cur[:, base + d: base + d + 512],
                                                 start=False, stop=False,
                                                 skip_group_check=True)
                            nc.tensor.matmul(ps[:, po:po + 512],
                                             mt[s][:, 2 * P:3 * P],
                                             rb[s][:, W * b + po:
                                                   W * b + po + 512],
                                             start=False, stop=True,
                                             skip_group_check=True)
                        # evac whole block: nxt = (A2 * -f2) + psum
                        p0, psz = (96, 30) if b == NBLK - 1 else (0, 126)
                        nc.vector.scalar_tensor_tensor(
                            nxt[p0:p0 + psz, bof + 4: bof + 1024],
                            a2[p0:p0 + psz, 0:1020],
                            cf[s][p0:p0 + psz, 0:1],
                            ps[p0:p0 + psz, 2:1022],
                            op0=mybir.AluOpType.mult,
                            op1=mybir.AluOpType.add,
                        )
                    # overlap-row maintenance for next sweep (batched)
                    v = nxt[:].rearrange("p (b w) -> p b w", b=NBLK)
                    nc.sync.dma_start(v[0:2, 1:8, :], v[124:126, 0:7, :])
                    nc.scalar.dma_start(v[126:128, 0:7, :], v[2:4, 1:8, :])
                    nc.sync.dma_start(
                        nxt[96:98, BW * 8: BW * 9], nxt[124:126, BW * 7: BW * 8])
                    nc.scalar.dma_start(
                        nxt[126:128, BW * 7: BW * 8], nxt[98:100, BW * 8: BW * 9])
                    # restore fixed global rows 0,1 (block 0)
                    nc.scalar.dma_start(nxt[0:2, 0:BW], cur[0:2, 0:BW])

            for s in range(SPC):
                final = gb[s][N_ITER % 2]
                for lo, hi in ((0, 3), (3, 6), (6, NBLK)):
                    nc.sync.dma_start(o_d[s][:, BW * lo: BW * hi],
                                      final[:, BW * lo: BW * hi])

    nc.compile()
    return nc


def _get_nc():
    if "nc" not in _CACHE:
        _CACHE["nc"] = _build_nc()
    return _CACHE["nc"]


def _to_blocks(x, width, guard):
    """[B, H, W(+0)] f32 -> [B, P, NBLK*(W+2*guard)] bf16 with row overlap."""
    nb = x.shape[0]
    out = np.zeros((nb, P, NBLK * (width + 2 * guard)), BF)
    for b in range(NBLK):
        rs = _row_start(b)
        sl = out[:, :, b * (width + 2 * guard) + guard:
                 (b + 1) * (width + 2 * guard) - guard]
        sl[:] = x[:, rs:rs + P, :].astype(BF)
    return out


def kernel(current_guess, rhses, dx):
    from concourse.bass_utils import run_bass_kernel_spmd

    g32 = np.ascontiguousarray(current_guess[:, 0], dtype=np.float32)
    r32 = np.ascontiguousarray(rhses[:, 0], dtype=np.float32)
    mats, scal, dinv = _host_mats(dx)
    g = _to_blocks(g32, W, 2)
    r = _to_blocks(r32 * dinv[:, None, None].astype(np.float32), W, 0)

    nc = _get_nc()
    in_maps = []
    for c in range(N_CORES):
        sl = slice(c * SPC, (c + 1) * SPC)
        in_maps.append({
            "g": np.ascontiguousarray(g[sl]).view(np.uint16),
            "r": np.ascontiguousarray(r[sl]).view(np.uint16),
            "m": np.ascontiguousarray(mats[sl]).view(np.uint16),
            "c": np.ascontiguousarray(scal[sl]),
        })
    res = run_bass_kernel_spmd(nc, in_maps, core_ids=list(range(N_CORES)))
    _CACHE["last_results"] = res
    ob = np.concatenate([res.results[c]["o"] for c in range(N_CORES)], axis=0)
    blk = ob.view(BF).astype(np.float32).reshape(B, P, NBLK, BW).transpose(0, 2, 1, 3)

    out = np.empty((B, H, W), np.float32)
    for b in range(NBLK - 1):
        out[:, 124 * b + 2: 124 * b + 126, :] = blk[:, b, 2:126, 2:2 + W]
    out[:, 994:1022, :] = blk[:, NBLK - 1, 98:126, 2:2 + W]
    # exact fp32 boundary frame from the input
    out[:, 0:2, :] = g32[:, 0:2, :]
    out[:, 1022:1024, :] = g32[:, 1022:1024, :]
    out[:, :, 0:2] = g32[:, :, 0:2]
    out[:, :, 1022:1024] = g32[:, :, 1022:1024]
    return out[:, None].astype(np.float32)


# revision 15
# speedup vs baseline: 1.5514x; 1.0289x over previous
"""Trainium2 Bass kernel for 5-sweep Jacobi iteration (4th-order 2D Poisson).

Problem: B=16 samples of [1024,1024] f32; per-sample cross stencil from dx;
5 Jacobi sweeps; 2-wide boundary frame kept fixed at the initial guess.

Sharding: data-parallel over batch, 2 samples per core, 8 cores.

Layout: bf16 state, 9 row-blocks of 128 rows overlapping by 4 rows
(block b holds rows 124b..124b+128; block 8 holds rows 896..1024). Each
block computes out rows [2,126) locally (block 8: [98,126)) so the
H-direction taps never cross a block boundary -> no halo matmuls. The
4-row overlaps are kept coherent with small SBUF->SBUF DMAs per sweep.

Per [128,512] output unit and sweep:
  PE   : psum = Bc@x (H taps) + (-f1)(x<<1 + x>>1) + I@R     (4 matmuls)
  DVE/GPS: A2 = x<<2 + x>>2                                  (tensor_add)
  DVE  : nxt = (A2 * -f2) + psum                             (fused STT)
Boundary cols are never written (col-trimmed evac); boundary rows are
restored by DMA; host splices the exact fp32 boundary frame at the end.
"""

import sys

sys.path.insert(0, "/opt/trn_rl_repo")

import numpy as np
import ml_dtypes

BF = ml_dtypes.bfloat16

N_CORES = 8
B, H, W = 16, 1024, 1024
SPC = B // N_CORES  # samples per core
P = 128
OPB = 124            # out rows per block
NBLK = 9             # row blocks (8 full-stride + 1 tail)
BW = W + 4           # block width incl 2 guard cols each side
FREE = NBLK * BW     # 9252
RFREE = NBLK * W     # 9216
N_ITER = 5
GPS_FRAC_NUM, GPS_FRAC_DEN = 2, 5   # ~40% of A2 adds on gpsimd

_CACHE = {}


def _row_start(b):
    return 124 * b if b < NBLK - 1 else H - P  # block 8: rows 896..1024


def _host_coeffs(dx):
    """Per-sample stencil scalars in float64. dx: [B, 2]."""
    a = (1.0 / dx.astype(np.float64)) ** 2
    a0, a1 = a[:, 0], a[:, 1]
    dinv = 1.0 / (-2.5 * (a0 + a1))
    e1 = dinv * a0 * (4.0 / 3.0)
    e2 = dinv * a0 * (-1.0 / 12.0)
    f1 = dinv * a1 * (4.0 / 3.0)
    f2 = dinv * a1 * (-1.0 / 12.0)
    return dinv, e1, e2, f1, f2


def _host_mats(dx):
    """[B, 128, 384] lhsT mats: [Bc(-e taps) | -f1*I | I], plus -f2 scalars."""
    dinv, e1, e2, f1, f2 = _host_coeffs(dx)
    nb = dx.shape[0]
    mats = np.zeros((nb, P, 3 * P), np.float64)
    idx = np.arange(P)
    for s in range(nb):
        bc = mats[s, :, 0:P]
        for off, v in ((1, -e1[s]), (-1, -e1[s]), (2, -e2[s]), (-2, -e2[s])):
            kk = idx[(idx + off >= 0) & (idx + off < P)]
            bc[kk, kk + off] = v
        mats[s, :, P:2 * P][idx, idx] = -f1[s]
        mats[s, :, 2 * P:3 * P][idx, idx] = 1.0
    scal = np.broadcast_to((-f2)[:, None, None], (nb, P, 1))
    return mats.astype(BF), np.ascontiguousarray(scal, dtype=np.float32), dinv


def _build_nc():
    import concourse.bacc as bacc
    import concourse.tile as tile
    from concourse import mybir

    f32 = mybir.dt.float32
    bf16 = mybir.dt.bfloat16
    nc = bacc.Bacc(
        "TRN2",
        target_bir_lowering=False,
        debug=False,
        enable_asserts=False,
        num_devices=N_CORES,
    )
    g_d = nc.dram_tensor("g", [SPC, P, FREE], bf16, kind="ExternalInput").ap()
    r_d = nc.dram_tensor("r", [SPC, P, RFREE], bf16, kind="ExternalInput").ap()
    m_d = nc.dram_tensor("m", [SPC, P, 3 * P], bf16, kind="ExternalInput").ap()
    c_d = nc.dram_tensor("c", [SPC, P, 1], f32, kind="ExternalInput").ap()
    o_d = nc.dram_tensor("o", [SPC, P, FREE], bf16, kind="ExternalOutput").ap()

    with tile.TileContext(nc) as tc:
        with (
            tc.tile_pool(name="state", bufs=1) as state,
            tc.tile_pool(name="tmp", bufs=6) as tmp,
            tc.tile_pool(name="psum", bufs=4, space="PSUM") as pp,
        ):
            gb = [
                [state.tile([P, FREE], bf16, name=f"g{s}_{i}", tag=f"g{s}_{i}")
                 for i in range(2)]
                for s in range(SPC)
            ]
            rb = [state.tile([P, RFREE], bf16, name=f"r{s}", tag=f"r{s}")
                  for s in range(SPC)]
            mt = [state.tile([P, 3 * P], bf16, name=f"m{s}", tag=f"m{s}")
                  for s in range(SPC)]
            cf = [state.tile([P, 1], f32, name=f"c{s}", tag=f"c{s}")
                  for s in range(SPC)]

            from concourse.ap import AP

            for s in range(SPC):
                nc.sync.dma_start(mt[s][:], m_d[s])
                nc.sync.dma_start(cf[s][:], c_d[s])
            # spread block loads across trigger queues so the first blocks
            # land fast (each dma_start chain runs on one DMA engine)
            qs = [nc.sync, nc.scalar]
            qi = 0
            for s in range(SPC):
                for b in range(NBLK):
                    qs[qi % 2].dma_start(gb[s][0][:, BW * b: BW * (b + 1)],
                                         g_d[s][:, BW * b: BW * (b + 1)])
                    qs[(qi + 1) % 2].dma_start(rb[s][:, W * b: W * (b + 1)],
                                               r_d[s][:, W * b: W * (b + 1)])
                    qi += 1
            for s in range(SPC):
                # buffer 1 never gets a full load: evac rewrites everything
                # except the fixed boundary-col strips (offsets 2,3 / 1024,
                # 1025 per block) and block-8 rows 126,127.
                g0 = gb[s][0][:]
                g1 = gb[s][1][:]
                strips = [[9252, 128], [1028, NBLK], [1, 2]]
                for so in (2, 1024):
                    nc.sync.dma_start(
                        AP(tensor=g1.tensor, offset=g1.offset + so, ap=strips),
                        AP(tensor=g0.tensor, offset=g0.offset + so, ap=strips))
                # block 8 fully: partitions 0..95 are never evac'd and NaN
                # garbage there would poison the contraction
                nc.sync.dma_start(
                    gb[s][1][:, BW * (NBLK - 1): BW * NBLK],
                    gb[s][0][:, BW * (NBLK - 1): BW * NBLK])

            bidx = 0
            for it in range(N_ITER):
                for s in range(SPC):
                    cur = gb[s][it % 2]
                    nxt = gb[s][(it + 1) % 2]
                    for b in range(NBLK):
                        ps = pp.tile([P, 1024], f32, name="ps", tag="ps")
                        a2 = tmp.tile([P, 1020], bf16, name="a2", tag="a2")
                        bof = BW * b
                        # A2[n] = x[c-2]+x[c+2], c=n+2, on DVE or GPSIMD
                        eng = (nc.gpsimd if bidx % 9 in (1, 3, 5, 7, 8)
                               else nc.vector)
                        eng.tensor_add(a2[:], cur[:, bof + 2: bof + 1022],
                                       cur[:, bof + 6: bof + 1026])
                        bidx += 1
                        # PE: H banded + f1 shifts + R inject, per 512-half
                        for h2 in range(2):
                            base = bof + 2 + 512 * h2
                            po = 512 * h2
                            nc.tensor.matmul(ps[:, po:po + 512], mt[s][:, 0:P],
                                             cur[:, base: base + 512],
                                             start=True, stop=False,
                                             skip_group_check=True)
                            for d in (-1, 1):
                                nc.tensor.matmul(ps[:, po:po + 512],
                                                 mt[s][:, P:2 * P],
                                                 cur[:, base + d: base + d + 512],
                                                 start=False, stop=False,
                                                 skip_group_check=True)
                            nc.tensor.matmul(ps[:, po:po + 512],
                                             mt[s][:, 2 * P:3 * P],
                                             rb[s][:, W * b + po:
                                                   W * b + po + 512],
                                             start=False, stop=True,
                                             skip_group_check=True)
                        # evac whole block: nxt = (A2 * -f2) + psum
                        p0, psz = (96, 30) if b == NBLK - 1 else (0, 126)
                        nc.vector.scalar_tensor_tensor(
                            nxt[p0:p0 + psz, bof + 4: bof + 1024],
                            a2[p0:p0 + psz, 0:1020],
                            cf[s][p0:p0 + psz, 0:1],
                            ps[p0:p0 + psz, 2:1022],
                            op0=mybir.AluOpType.mult,
                            op1=mybir.AluOpType.add,
                        )
                    # overlap-row maintenance for next sweep (batched)
                    v = nxt[:].rearrange("p (b w) -> p b w", b=NBLK)
                    nc.sync.dma_start(v[0:2, 1:8, :], v[124:126, 0:7, :])
                    nc.scalar.dma_start(v[126:128, 0:7, :], v[2:4, 1:8, :])
                    nc.sync.dma_start(
                        nxt[96:98, BW * 8: BW * 9], nxt[124:126, BW * 7: BW * 8])
                    nc.scalar.dma_start(
                        nxt[126:128, BW * 7: BW * 8], nxt[98:100, BW * 8: BW * 9])
                    # restore fixed global rows 0,1 (block 0)
                    nc.scalar.dma_start(nxt[0:2, 0:BW], cur[0:2, 0:BW])

            for s in range(SPC):
                final = gb[s][N_ITER % 2]
                for lo, hi in ((0, 3), (3, 6), (6, NBLK)):
                    nc.sync.dma_start(o_d[s][:, BW * lo: BW * hi],
                                      final[:, BW * lo: BW * hi])

    nc.compile()
    return nc


def _get_nc():
    if "nc" not in _CACHE:
        _CACHE["nc"] = _build_nc()
    return _CACHE["nc"]


def _to_blocks(x, width, guard):
    """[B, H, W(+0)] f32 -> [B, P, NBLK*(W+2*guard)] bf16 with row overlap."""
    nb = x.shape[0]
    out = np.zeros((nb, P, NBLK * (width + 2 * guard)), BF)
    for b in range(NBLK):
        rs = _row_start(b)
        sl = out[:, :, b * (width + 2 * guard) + guard:
                 (b + 1) * (width + 2 * guard) - guard]
        sl[:] = x[:, rs:rs + P, :].astype(BF)
    return out


def kernel(current_guess, rhses, dx):
    from concourse.bass_utils import run_bass_kernel_spmd

    g32 = np.ascontiguousarray(current_guess[:, 0], dtype=np.float32)
    r32 = np.ascontiguousarray(rhses[:, 0], dtype=np.float32)
    mats, scal, dinv = _host_mats(dx)
    g = _to_blocks(g32, W, 2)
    r = _to_blocks(r32 * dinv[:, None, None].astype(np.float32), W, 0)

    nc = _get_nc()
    in_maps = []
    for c in range(N_CORES):
        sl = slice(c * SPC, (c + 1) * SPC)
        in_maps.append({
            "g": np.ascontiguousarray(g[sl]).view(np.uint16),
            "r": np.ascontiguousarray(r[sl]).view(np.uint16),
            "m": np.ascontiguousarray(mats[sl]).view(np.uint16),
            "c": np.ascontiguousarray(scal[sl]),
        })
    res = run_bass_kernel_spmd(nc, in_maps, core_ids=list(range(N_CORES)))
    _CACHE["last_results"] = res
    ob = np.concatenate([res.results[c]["o"] for c in range(N_CORES)], axis=0)
    blk = ob.view(BF).astype(np.float32).reshape(B, P, NBLK, BW).transpose(0, 2, 1, 3)

    out = np.empty((B, H, W), np.float32)
    for b in range(NBLK - 1):
        out[:, 124 * b + 2: 124 * b + 126, :] = blk[:, b, 2:126, 2:2 + W]
    out[:, 994:1022, :] = blk[:, NBLK - 1, 98:126, 2:2 + W]
    # exact fp32 boundary frame from the input
    out[:, 0:2, :] = g32[:, 0:2, :]
    out[:, 1022:1024, :] = g32[:, 1022:1024, :]
    out[:, :, 0:2] = g32[:, :, 0:2]
    out[:, :, 1022:1024] = g32[:, :, 1022:1024]
    return out[:, None].astype(np.float32)
